# revision 27
# baseline (speedup 1.0000x reference)
"""Trainium2 Bass kernel for CUDALinearAttention (b=4, t=4096, d=1024, h=16).

Sharding: 8 NeuronCores = 4 batches x 2 head-groups (8 heads / 512 out-dims each).
Each core is fully independent (KV aggregation is per-head); no collectives.

Per-core pipeline (all matmuls bf16, fp32 PSUM accumulation), per t-quarter:
  T: x loaded token-major (split DMAs), cast to bf16 (DVE), transposed to xT
     (d-on-partitions) via PE identity-matmul (8 blocks per PSUM bank) or DMA
     xbar transpose (LK_TMODE=dma).
  A: k/v projections token-major; phi(x)=min(exp(x),1)+relu(x) (exp on ACT
     straight from PSUM, clamp on GpSimd, assemble on DVE), mask folded in;
     v stored per-pair as [v_h0 | m | v_h1 | m] (130-wide blocks).
  B: per head pair j one matmul chain over the quarter's t: lhsT = kf pair
     cols [128,128], rhs = va pair block [128,130] -> kv of both heads in
     row-halves, z in col 64 (garbage halves never read); accumulated across
     quarters into SBUF (kvs32), so the small-matmul work stays inside the
     dense (HAM-warm) region.
  C: q projection head-major (W stationary, xT moving): qfT[o,t] -- already
     K(=hd)-major for num/den.
Then kvs32 is finalized zero-padded into kvs (so downstream matmuls contract
K=128 from base partition 0; row-group-64 operands crash hardware), and
  D: one matmul per pair/chunk: rhs = kvs[:,j,:] = [kv_h0|kv_h1|z0|z1]
     [128,130] -> cols 0..127 = num both heads, 128/129 = den; then
     out = num * recip(max(den,1e-6)) * mask on DVE; DMA out.
"""

import os
import sys

sys.path.insert(0, "/opt/trn_rl_repo")

import numpy as np
import ml_dtypes

import concourse.bass as bass
import concourse.tile as tile
from concourse import bacc, mybir
from concourse.bass_utils import run_bass_kernel_spmd
from concourse.masks import make_identity

F32 = mybir.dt.float32
BF16 = mybir.dt.bfloat16
AF = mybir.ActivationFunctionType
ALU = mybir.AluOpType

T = 4096
D = 1024
HG = 512  # per-core output dims (8 heads x 64)
KC = 8  # contraction chunks of 128 over D
TC = 32  # token chunks of 128
OC = 4  # output-dim chunks of 128 within HG (= head pairs)
HALVES = 4  # t mega-chunks (xT quarter double-buffered)
TCH = TC // HALVES
T5H = (T // 512) // HALVES


def _build_program(has_bias: bool, has_mask: bool):
    stages = os.environ.get("LK_STAGES", "TABCD")
    tmode = os.environ.get("LK_TMODE", "pe")
    nc = bacc.Bacc("TRN2", target_bir_lowering=False, debug=False)

    xb = nc.dram_tensor("xb", [T, D], F32, kind="ExternalInput")
    maskb = nc.dram_tensor("maskb", [T], F32, kind="ExternalInput")
    wqt = nc.dram_tensor("wqt", [D, HG], BF16, kind="ExternalInput")
    wkt = nc.dram_tensor("wkt", [D, HG], BF16, kind="ExternalInput")
    wvt = nc.dram_tensor("wvt", [D, HG], BF16, kind="ExternalInput")
    bqp = nc.dram_tensor("bqp", [HG], F32, kind="ExternalInput")
    bkr = nc.dram_tensor("bkr", [1, HG], BF16, kind="ExternalInput")
    bvr = nc.dram_tensor("bvr", [1, HG], BF16, kind="ExternalInput")
    outd = nc.dram_tensor("out", [T, HG], F32, kind="ExternalOutput")

    with tile.TileContext(nc) as tc:
        with (
            tc.tile_pool(name="const", bufs=1) as constp,
            tc.tile_pool(name="wp", bufs=1) as wp,
            tc.tile_pool(name="xTp", bufs=2) as xTp,
            tc.tile_pool(name="kfp", bufs=1) as kfp,
            tc.tile_pool(name="vap", bufs=1) as vap,
            tc.tile_pool(name="qfp", bufs=1) as qfp,
            tc.tile_pool(name="kvsp", bufs=1) as kvsp,
            tc.tile_pool(name="stage", bufs=4) as stage,
            tc.tile_pool(name="ptmp", bufs=3) as ptmp,
            tc.tile_pool(name="outp", bufs=3) as outp,
            tc.tile_pool(name="rdp", bufs=2) as rdp,
            tc.tile_pool(
                name="projp", bufs=4 if tmode != "pe" else 3, space="PSUM"
            ) as projp,
            tc.tile_pool(
                name="nmp", bufs=4 if tmode != "pe" else 3, space="PSUM"
            ) as nmp,
        ):
            tpsp_cm = None
            tpsp = None
            if tmode == "pe":
                tpsp_cm = tc.tile_pool(name="tpsp", bufs=2, space="PSUM")
                tpsp = tpsp_cm.__enter__()

            # pre-issue the first x tiles before anything else (startup latency)
            xs_pre = []
            for i in range(4):
                xsp = stage.tile([128, D], F32, tag="xs")
                r = slice(i * 128, (i + 1) * 128)
                nc.sync.dma_start(xsp[:, 0:512], xb.ap()[r, 0:512])
                nc.sync.dma_start(xsp[:, 512:1024], xb.ap()[r, 512:1024])
                xs_pre.append(xsp)

            # ---- constants ----
            ident = constp.tile([128, 128], BF16)
            make_identity(nc, ident[:])
            mask_sb = constp.tile([128, TC], F32)
            nc.sync.dma_start(mask_sb[:], maskb.ap().rearrange("(a p) -> p a", p=128))
            bq_sb = constp.tile([128, OC], F32)
            nc.sync.dma_start(bq_sb[:], bqp.ap().rearrange("(a p) -> p a", p=128))
            eps_sb = constp.tile([128, 1], F32)
            nc.vector.memset(eps_sb[:], 1e-6)
            if has_bias:
                ones_b = constp.tile([1, 128], BF16)
                nc.vector.memset(ones_b[:], 1.0)
                bk_sb = constp.tile([1, HG], BF16)
                nc.sync.dma_start(bk_sb[:], bkr.ap())
                bv_sb = constp.tile([1, HG], BF16)
                nc.sync.dma_start(bv_sb[:], bvr.ap())

            # ---- weights (host pre-transposed to [D, HG]); DMAs emitted
            # lazily so the first x tiles win the DMA queues ----
            w_sb = {}
            w_dram = {"q": wqt, "k": wkt, "v": wvt}

            def load_w(name):
                if name not in w_sb:
                    w = wp.tile([128, KC, HG], BF16, tag=f"w{name}")
                    nc.sync.dma_start(
                        w[:], w_dram[name].ap().rearrange("(kc p) n -> p kc n", p=128)
                    )
                    w_sb[name] = w
                return w_sb[name]

            # ---- big persistent activations ----
            kf = kfp.tile([128, TC, HG], BF16)
            va = vap.tile([128, TC, OC * 130], BF16)
            qf = qfp.tile([128, OC, T], BF16)
            kvs32 = kvsp.tile([128, OC, 130], F32, tag="kvs32")
            nc.vector.memset(kvs32[:], 0.0)
            # kvs[:, j, :] = [kv_h0 (rows 0-63) | kv_h1 (rows 64-127) | z0 | z1],
            # complementary rows zero
            kvs = kvsp.tile([128, OC, 130], BF16)
            nc.vector.memset(kvs[:], 0.0)
            if not has_mask:
                # ones columns of va are the constant 1.0 mask; set once
                va_ones = va[:].rearrange("p t (j h c) -> p t j h c", h=2, c=65)
                nc.vector.memset(va_ones[:, :, :, :, 64:65], 1.0)

            for half in range(HALVES):
                xT = xTp.tile([128, KC, T // HALVES], BF16, tag="xT")

                # ---- phase T: load + cast + transpose ----
                for tl in range(TCH if "T" in stages else 0):
                    t_c = half * TCH + tl
                    if t_c < 4:
                        xs = xs_pre[t_c]
                    else:
                        xs = stage.tile([128, D], F32, tag="xs")
                        r = slice(t_c * 128, (t_c + 1) * 128)
                        nc.sync.dma_start(xs[:], xb.ap()[r, :])
                    xc = stage.tile([128, D], BF16, tag="xc")
                    nc.vector.tensor_copy(xc[:, 0:512], xs[:, 0:512])
                    nc.vector.tensor_copy(xc[:, 512:1024], xs[:, 512:1024])
                    if tmode == "dma1":
                        nc.sync.dma_start_transpose(
                            xT[:, :, tl * 128 : (tl + 1) * 128], xc[:]
                        )
                    elif tmode == "dma":
                        for kc in range(KC):
                            nc.sync.dma_start_transpose(
                                xT[:, kc, tl * 128 : (tl + 1) * 128],
                                xc[:, kc * 128 : (kc + 1) * 128],
                            )
                    else:
                        tp = tpsp.tile([128, KC, 128], BF16, tag="tps")
                        for kc in range(KC):
                            nc.tensor.matmul(
                                tp[:, kc, :],
                                xc[:, kc * 128 : (kc + 1) * 128],
                                ident[:],
                                is_transpose=True,
                                start=(kc == 0),
                                stop=(kc == KC - 1),
                            )
                        dst = xT[:, :, tl * 128 : (tl + 1) * 128]
                        if tl % 2 == 0:
                            nc.vector.tensor_copy(dst, tp[:])
                        else:
                            nc.scalar.copy(dst, tp[:])

                # ---- phase A: k/v projections (token-major) + phi/mask ----
                for tl in range(TCH if "A" in stages else 0):
                    t_c = half * TCH + tl
                    m_col = mask_sb[:, t_c : t_c + 1]

                    kp = projp.tile([128, 512], F32, tag="big")
                    for kc in range(KC):
                        nc.tensor.matmul(
                            kp[:],
                            xT[:, kc, tl * 128 : (tl + 1) * 128],
                            load_w("k")[:, kc, :],
                            start=(kc == 0),
                            stop=(kc == KC - 1 and not has_bias),
                        )
                    if has_bias:
                        nc.tensor.matmul(
                            kp[:], ones_b[:], bk_sb[:], start=False, stop=True
                        )
                    # phi(x) = min(exp(x), 1) + relu(x); exp is safe: min(inf,1)=1
                    ke = ptmp.tile([128, 512], F32, tag="ex")
                    nc.scalar.activation(ke[:], kp[:], AF.Exp)
                    nc.vector.tensor_scalar_min(ke[:], ke[:], 1.0)
                    kr = ptmp.tile([128, 512], F32, tag="rl")
                    if has_mask:
                        # relu(k * m) == m * relu(k) for m >= 0
                        nc.scalar.activation(kr[:], kp[:], AF.Relu, scale=m_col)
                        nc.vector.scalar_tensor_tensor(
                            kf[:, t_c, :], ke[:], m_col, kr[:],
                            op0=ALU.mult, op1=ALU.add,
                        )
                    else:
                        nc.scalar.activation(kr[:], kp[:], AF.Relu)
                        nc.vector.tensor_add(kf[:, t_c, :], ke[:], kr[:])

                    vp = projp.tile([128, 512], F32, tag="big")
                    for kc in range(KC):
                        nc.tensor.matmul(
                            vp[:],
                            xT[:, kc, tl * 128 : (tl + 1) * 128],
                            load_w("v")[:, kc, :],
                            start=(kc == 0),
                            stop=(kc == KC - 1 and not has_bias),
                        )
                    if has_bias:
                        nc.tensor.matmul(
                            vp[:], ones_b[:], bv_sb[:], start=False, stop=True
                        )
                    va_t = va[:, t_c, :].rearrange("p (j h c) -> p j h c", h=2, c=65)
                    vp_t = vp[:].rearrange("p (j h c) -> p j h c", h=2, c=64)
                    if has_mask:
                        nc.scalar.mul(va_t[:, :, :, 0:64], vp_t, m_col)
                        nc.vector.tensor_copy(
                            va_t[:, :, :, 64:65], m_col.broadcast_to((128, OC, 2, 1))
                        )
                    else:
                        nc.scalar.copy(va_t[:, :, :, 0:64], vp_t)

                # ---- phase B: per-pair KV partial accumulation (this quarter) ----
                for j in range(OC if "B" in stages else 0):
                    kvp_t2 = nmp.tile([128, 2, 130], F32, tag="nm")
                    kvp_t = kvp_t2[:, 0, :]
                    for tl in range(TCH):
                        t_c = half * TCH + tl
                        nc.tensor.matmul(
                            kvp_t[:],
                            kf[:, t_c, j * 128 : (j + 1) * 128],
                            va[:, t_c, j * 130 : (j + 1) * 130],
                            start=(tl == 0),
                            stop=(tl == TCH - 1),
                        )
                    nc.vector.tensor_add(kvs32[:, j, :], kvs32[:, j, :], kvp_t[:])

                # ---- phase C: q projection (head-major) + phi ----
                def emit_C(half, t5l, oc, xT=None):
                    t5 = half * T5H + t5l
                    qp = projp.tile([128, 512], F32, tag="big")
                    for kc in range(KC):
                        nc.tensor.matmul(
                            qp[:],
                            load_w("q")[:, kc, oc * 128 : (oc + 1) * 128],
                            xT[:, kc, t5l * 512 : (t5l + 1) * 512],
                            start=(kc == 0),
                            stop=(kc == KC - 1),
                        )
                    b_col = bq_sb[:, oc : oc + 1]
                    qe = ptmp.tile([128, 512], F32, tag="ex")
                    qr = ptmp.tile([128, 512], F32, tag="rl")
                    if has_bias:
                        nc.scalar.activation(qe[:], qp[:], AF.Exp, bias=b_col)
                        nc.scalar.activation(qr[:], qp[:], AF.Relu, bias=b_col)
                    else:
                        nc.scalar.activation(qe[:], qp[:], AF.Exp)
                        nc.scalar.activation(qr[:], qp[:], AF.Relu)
                    nc.vector.tensor_scalar_min(qe[:], qe[:], 1.0)
                    nc.vector.tensor_add(
                        qf[:, oc, t5 * 512 : (t5 + 1) * 512], qe[:], qr[:]
                    )

                if "C" in stages and half < HALVES - 1:
                    for t5l in range(T5H):
                        for oc in range(OC):
                            emit_C(half, t5l, oc, xT=xT)
                else:
                    last_xT = xT

            # ---- finalize kvs (bf16, zero-padded) from kvs32 ----
            if "B" in stages:
                for j in range(OC):
                    kj = kvs32[:, j, :]
                    nc.vector.tensor_copy(kvs[0:64, j, 0:64], kj[0:64, 0:64])
                    nc.vector.tensor_copy(kvs[0:64, j, 128:129], kj[0:64, 64:65])
                    nc.vector.tensor_copy(kvs[64:128, j, 64:128], kj[64:128, 65:129])
                    nc.vector.tensor_copy(kvs[64:128, j, 129:130], kj[64:128, 64:65])

            # ---- phase D: fused num+den + normalize + store ----
            warm = os.environ.get("LK_WARM", "1") == "1"

            def emit_D(t_c, extra_warm=False):
                m_col = mask_sb[:, t_c : t_c + 1]
                if warm:
                    # dense dummy matmul keeps the PE clock-gate at 8/8
                    # through the small-matmul tail (projp is idle here)
                    for _ in range(2 if extra_warm else 1):
                        wp_t = projp.tile([128, 512], F32, tag="big")
                        nc.tensor.matmul(
                            wp_t[:], xT[:, 0, 0:128], load_w("k")[:, 0, :],
                            start=True, stop=True, skip_group_check=True,
                        )
                nms = []
                for jj in range(2):  # two pairs per PSUM bank tile
                    nm2 = nmp.tile([128, 2, 130], F32, tag="nm")
                    for j2 in range(2):
                        nc.tensor.matmul(
                            nm2[:, j2, :],
                            qf[:, jj * 2 + j2, t_c * 128 : (t_c + 1) * 128],
                            kvs[:, jj * 2 + j2, :],
                            start=True,
                            stop=True,
                        )
                    nms.append(nm2)
                rden = rdp.tile([128, 8], F32, tag="rd")
                # strided read gathers den cols; max applies the clamp
                nc.vector.tensor_scalar_max(
                    rden[:].rearrange("p (a b) -> p a b", a=2)[:, 0],
                    nms[0][:, :, 128:130],
                    1e-6,
                )
                nc.scalar.activation(
                    rden[:].rearrange("p (a b) -> p a b", a=2)[:, 1],
                    nms[1][:, :, 128:130],
                    AF.Relu,
                    bias=eps_sb[:],
                )
                nc.vector.reciprocal(rden[:], rden[:])
                if has_mask:
                    nc.vector.tensor_scalar_mul(rden[:], rden[:], m_col)
                # two wide muls; recip fed via step-0 broadcast AP (no expand)
                ot = outp.tile([128, HG], F32, tag="ot")
                for jj in range(2):
                    nc.vector.tensor_mul(
                        ot[:, jj * 256 : (jj + 1) * 256].rearrange(
                            "p (a b c) -> p a b c", b=2, c=64
                        ),
                        nms[jj][:, :, 0:128].rearrange("p a (b c) -> p a b c", c=64),
                        rden[:, jj * 4 : (jj + 1) * 4]
                        .rearrange("p (a b) -> p a b", b=2)
                        .unsqueeze(-1)
                        .broadcast_to((128, 2, 2, 64)),
                    )
                nc.sync.dma_start(outd.ap()[t_c * 128 : (t_c + 1) * 128, :], ot[:])

            if "D" in stages:
                d_order = []
                if "C" in stages:
                    # interleave last-quarter q-projection across early D chunks
                    # (dense N=512 bursts keep the HAM clock-gate warm)
                    dq = list(range((HALVES - 1) * TCH))
                    for t5l in range(T5H):
                        for oc in range(OC):
                            d_order.append(("C", t5l, oc))
                            for _ in range(3):
                                if dq:
                                    d_order.append(("D", dq.pop(0), None))
                    for t_c in dq:
                        d_order.append(("D", t_c, None))
                    for t_c in range((HALVES - 1) * TCH, TC):
                        d_order.append(("D", t_c, None))
                else:
                    d_order = [("D", t_c, None) for t_c in range(TC)]
                n_c_left = sum(1 for k, _, _ in d_order if k == "C")
                for kind, a, b2 in d_order:
                    if kind == "C":
                        emit_C(HALVES - 1, a, b2, xT=last_xT)
                        n_c_left -= 1
                    else:
                        emit_D(a, extra_warm=(n_c_left == 0))

            if tpsp_cm is not None:
                tpsp_cm.__exit__(None, None, None)

    nc.compile()
    return nc


_PROGRAM_CACHE = {}


def _get_program(has_bias: bool, has_mask: bool):
    key = (has_bias, has_mask)
    if key not in _PROGRAM_CACHE:
        _PROGRAM_CACHE[key] = _build_program(has_bias, has_mask)
    return _PROGRAM_CACHE[key]


def _prep_inputs(x, mask, Wq, bq, Wk, bk, Wv, bv):
    """Slice + lay out per-core inputs. Core c -> batch c//2, head-group c%2."""
    bf16 = ml_dtypes.bfloat16
    in_maps = []
    for c in range(8):
        bi, hg = c // 2, c % 2
        sl = slice(hg * HG, (hg + 1) * HG)
        in_maps.append(
            {
                "xb": np.ascontiguousarray(x[bi]).astype(np.float32, copy=False),
                "maskb": np.ascontiguousarray(mask[bi]).astype(np.float32, copy=False),
                "wqt": np.ascontiguousarray(Wq[sl, :].T).astype(bf16),
                "wkt": np.ascontiguousarray(Wk[sl, :].T).astype(bf16),
                "wvt": np.ascontiguousarray(Wv[sl, :].T).astype(bf16),
                "bqp": np.ascontiguousarray(bq[sl]).astype(np.float32, copy=False),
                "bkr": np.ascontiguousarray(bk[sl]).astype(bf16).reshape(1, HG),
                "bvr": np.ascontiguousarray(bv[sl]).astype(bf16).reshape(1, HG),
            }
        )
    return in_maps


def kernel(x, mask, Wq, bq, Wk, bk, Wv, bv, n_heads, **run_kwargs):
    x = np.asarray(x)
    mask = np.asarray(mask)
    Wq, bq = np.asarray(Wq), np.asarray(bq)
    Wk, bk = np.asarray(Wk), np.asarray(bk)
    Wv, bv = np.asarray(Wv), np.asarray(bv)
    b, t, d = x.shape
    assert (b, t, d) == (4, T, D) and int(n_heads) == 16, (
        f"kernel hardcoded for (4,{T},{D}) h=16, got {(b, t, d)} h={n_heads}"
    )

    has_bias = bool(np.any(bq) or np.any(bk) or np.any(bv))
    has_mask = not bool(np.all(mask == 1.0))
    nc = _get_program(has_bias, has_mask)
    in_maps = _prep_inputs(x, mask, Wq, bq, Wk, bk, Wv, bv)
    res = run_bass_kernel_spmd(nc, in_maps, core_ids=list(range(8)), **run_kwargs)

    out = np.empty((4, T, D), dtype=np.float32)
    for c in range(8):
        bi, hg = c // 2, c % 2
        out[bi, :, hg * HG : (hg + 1) * HG] = res.results[c]["out"]
    if run_kwargs:
        kernel.last_results = res
    return out


# revision 28
# speedup vs baseline: 1.0060x; 1.0060x over previous
"""Trainium2 Bass kernel for CUDALinearAttention (b=4, t=4096, d=1024, h=16).

Sharding: 8 NeuronCores = 4 batches x 2 head-groups (8 heads / 512 out-dims each).
Each core is fully independent (KV aggregation is per-head); no collectives.

Per-core pipeline (all matmuls bf16, fp32 PSUM accumulation), per t-quarter:
  T: x loaded token-major (split DMAs), cast to bf16 (DVE), transposed to xT
     (d-on-partitions) via PE identity-matmul (8 blocks per PSUM bank) or DMA
     xbar transpose (LK_TMODE=dma).
  A: k/v projections token-major; phi(x)=min(exp(x),1)+relu(x) (exp on ACT
     straight from PSUM, clamp on GpSimd, assemble on DVE), mask folded in;
     v stored per-pair as [v_h0 | m | v_h1 | m] (130-wide blocks).
  B: per head pair j one matmul chain over the quarter's t: lhsT = kf pair
     cols [128,128], rhs = va pair block [128,130] -> kv of both heads in
     row-halves, z in col 64 (garbage halves never read); accumulated across
     quarters into SBUF (kvs32), so the small-matmul work stays inside the
     dense (HAM-warm) region.
  C: q projection head-major (W stationary, xT moving): qfT[o,t] -- already
     K(=hd)-major for num/den.
Then kvs32 is finalized zero-padded into kvs (so downstream matmuls contract
K=128 from base partition 0; row-group-64 operands crash hardware), and
  D: one matmul per pair/chunk: rhs = kvs[:,j,:] = [kv_h0|kv_h1|z0|z1]
     [128,130] -> cols 0..127 = num both heads, 128/129 = den; then
     out = num * recip(max(den,1e-6)) * mask on DVE; DMA out.
"""

import os
import sys

sys.path.insert(0, "/opt/trn_rl_repo")

import numpy as np
import ml_dtypes

import concourse.bass as bass
import concourse.tile as tile
from concourse import bacc, mybir
from concourse.bass_utils import run_bass_kernel_spmd
from concourse.masks import make_identity

F32 = mybir.dt.float32
BF16 = mybir.dt.bfloat16
AF = mybir.ActivationFunctionType
ALU = mybir.AluOpType

T = 4096
D = 1024
HG = 512  # per-core output dims (8 heads x 64)
KC = 8  # contraction chunks of 128 over D
TC = 32  # token chunks of 128
OC = 4  # output-dim chunks of 128 within HG (= head pairs)
HALVES = 4  # t mega-chunks (xT quarter double-buffered)
TCH = TC // HALVES
T5H = (T // 512) // HALVES


def _build_program(has_bias: bool, has_mask: bool):
    stages = os.environ.get("LK_STAGES", "TABCD")
    tmode = os.environ.get("LK_TMODE", "pe")
    nc = bacc.Bacc("TRN2", target_bir_lowering=False, debug=False)

    xb = nc.dram_tensor("xb", [T, D], F32, kind="ExternalInput")
    maskb = nc.dram_tensor("maskb", [T], F32, kind="ExternalInput")
    wqt = nc.dram_tensor("wqt", [D, HG], BF16, kind="ExternalInput")
    wkt = nc.dram_tensor("wkt", [D, HG], BF16, kind="ExternalInput")
    wvt = nc.dram_tensor("wvt", [D, HG], BF16, kind="ExternalInput")
    bqp = nc.dram_tensor("bqp", [HG], F32, kind="ExternalInput")
    bkr = nc.dram_tensor("bkr", [1, HG], BF16, kind="ExternalInput")
    bvr = nc.dram_tensor("bvr", [1, HG], BF16, kind="ExternalInput")
    outd = nc.dram_tensor("out", [T, HG], F32, kind="ExternalOutput")

    with tile.TileContext(nc) as tc:
        with (
            tc.tile_pool(name="const", bufs=1) as constp,
            tc.tile_pool(name="wp", bufs=1) as wp,
            tc.tile_pool(name="xTp", bufs=2) as xTp,
            tc.tile_pool(name="kfp", bufs=1) as kfp,
            tc.tile_pool(name="vap", bufs=1) as vap,
            tc.tile_pool(name="qfp", bufs=1) as qfp,
            tc.tile_pool(name="kvsp", bufs=1) as kvsp,
            tc.tile_pool(name="stage", bufs=4) as stage,
            tc.tile_pool(name="ptmp", bufs=3) as ptmp,
            tc.tile_pool(name="outp", bufs=3) as outp,
            tc.tile_pool(name="rdp", bufs=2) as rdp,
            tc.tile_pool(
                name="projp", bufs=4 if tmode != "pe" else 3, space="PSUM"
            ) as projp,
            tc.tile_pool(
                name="nmp", bufs=4 if tmode != "pe" else 3, space="PSUM"
            ) as nmp,
        ):
            tpsp_cm = None
            tpsp = None
            if tmode == "pe":
                tpsp_cm = tc.tile_pool(name="tpsp", bufs=2, space="PSUM")
                tpsp = tpsp_cm.__enter__()

            # pre-issue the first x tiles before anything else (startup latency)
            xs_pre = []
            for i in range(4):
                xsp = stage.tile([128, D], F32, tag="xs")
                r = slice(i * 128, (i + 1) * 128)
                nc.sync.dma_start(xsp[:, 0:512], xb.ap()[r, 0:512])
                nc.sync.dma_start(xsp[:, 512:1024], xb.ap()[r, 512:1024])
                xs_pre.append(xsp)

            # ---- constants ----
            ident = constp.tile([128, 128], BF16)
            make_identity(nc, ident[:])
            mask_sb = constp.tile([128, TC], F32)
            nc.sync.dma_start(mask_sb[:], maskb.ap().rearrange("(a p) -> p a", p=128))
            bq_sb = constp.tile([128, OC], F32)
            nc.sync.dma_start(bq_sb[:], bqp.ap().rearrange("(a p) -> p a", p=128))
            eps_sb = constp.tile([128, 1], F32)
            nc.vector.memset(eps_sb[:], 1e-6)
            if has_bias:
                ones_b = constp.tile([1, 128], BF16)
                nc.vector.memset(ones_b[:], 1.0)
                bk_sb = constp.tile([1, HG], BF16)
                nc.sync.dma_start(bk_sb[:], bkr.ap())
                bv_sb = constp.tile([1, HG], BF16)
                nc.sync.dma_start(bv_sb[:], bvr.ap())

            # ---- weights (host pre-transposed to [D, HG]); DMAs emitted
            # lazily so the first x tiles win the DMA queues ----
            w_sb = {}
            w_dram = {"q": wqt, "k": wkt, "v": wvt}

            def load_w(name):
                if name not in w_sb:
                    w = wp.tile([128, KC, HG], BF16, tag=f"w{name}")
                    nc.sync.dma_start(
                        w[:], w_dram[name].ap().rearrange("(kc p) n -> p kc n", p=128)
                    )
                    w_sb[name] = w
                return w_sb[name]

            # ---- big persistent activations ----
            kf = kfp.tile([128, TC, HG], BF16)
            va = vap.tile([128, TC, OC * 130], BF16)
            qf = qfp.tile([128, OC, T], BF16)
            kvs32 = kvsp.tile([128, OC, 130], F32, tag="kvs32")
            nc.vector.memset(kvs32[:], 0.0)
            # kvs[:, j, :] = [kv_h0 (rows 0-63) | kv_h1 (rows 64-127) | z0 | z1],
            # complementary rows zero
            kvs = kvsp.tile([128, OC, 130], BF16)
            nc.vector.memset(kvs[:], 0.0)
            if not has_mask:
                # ones columns of va are the constant 1.0 mask; set once
                va_ones = va[:].rearrange("p t (j h c) -> p t j h c", h=2, c=65)
                nc.vector.memset(va_ones[:, :, :, :, 64:65], 1.0)

            for half in range(HALVES):
                xT = xTp.tile([128, KC, T // HALVES], BF16, tag="xT")

                # ---- phase T: load + cast + transpose ----
                for tl in range(TCH if "T" in stages else 0):
                    t_c = half * TCH + tl
                    if t_c < 4:
                        xs = xs_pre[t_c]
                    else:
                        xs = stage.tile([128, D], F32, tag="xs")
                        r = slice(t_c * 128, (t_c + 1) * 128)
                        nc.sync.dma_start(xs[:], xb.ap()[r, :])
                    xc = stage.tile([128, D], BF16, tag="xc")
                    nc.vector.tensor_copy(xc[:, 0:512], xs[:, 0:512])
                    nc.vector.tensor_copy(xc[:, 512:1024], xs[:, 512:1024])
                    if tmode == "dma1":
                        nc.sync.dma_start_transpose(
                            xT[:, :, tl * 128 : (tl + 1) * 128], xc[:]
                        )
                    elif tmode == "dma":
                        for kc in range(KC):
                            nc.sync.dma_start_transpose(
                                xT[:, kc, tl * 128 : (tl + 1) * 128],
                                xc[:, kc * 128 : (kc + 1) * 128],
                            )
                    else:
                        tp = tpsp.tile([128, KC, 128], BF16, tag="tps")
                        for kc in range(KC):
                            nc.tensor.matmul(
                                tp[:, kc, :],
                                xc[:, kc * 128 : (kc + 1) * 128],
                                ident[:],
                                is_transpose=True,
                                start=(kc == 0),
                                stop=(kc == KC - 1),
                            )
                        dst = xT[:, :, tl * 128 : (tl + 1) * 128]
                        if tl % 2 == 0:
                            nc.vector.tensor_copy(dst, tp[:])
                        else:
                            nc.scalar.copy(dst, tp[:])

                # ---- phase A: k/v projections (token-major) + phi/mask ----
                for tl in range(TCH if "A" in stages else 0):
                    t_c = half * TCH + tl
                    m_col = mask_sb[:, t_c : t_c + 1]

                    kp = projp.tile([128, 512], F32, tag="big")
                    for kc in range(KC):
                        nc.tensor.matmul(
                            kp[:],
                            xT[:, kc, tl * 128 : (tl + 1) * 128],
                            load_w("k")[:, kc, :],
                            start=(kc == 0),
                            stop=(kc == KC - 1 and not has_bias),
                        )
                    if has_bias:
                        nc.tensor.matmul(
                            kp[:], ones_b[:], bk_sb[:], start=False, stop=True
                        )
                    # phi(x) = min(exp(x), 1) + relu(x); exp is safe: min(inf,1)=1
                    ke = ptmp.tile([128, 512], F32, tag="ex")
                    nc.scalar.activation(ke[:], kp[:], AF.Exp)
                    nc.vector.tensor_scalar_min(ke[:], ke[:], 1.0)
                    kr = ptmp.tile([128, 512], F32, tag="rl")
                    if has_mask:
                        # relu(k * m) == m * relu(k) for m >= 0
                        nc.scalar.activation(kr[:], kp[:], AF.Relu, scale=m_col)
                        nc.vector.scalar_tensor_tensor(
                            kf[:, t_c, :], ke[:], m_col, kr[:],
                            op0=ALU.mult, op1=ALU.add,
                        )
                    else:
                        nc.scalar.activation(kr[:], kp[:], AF.Relu)
                        nc.vector.tensor_add(kf[:, t_c, :], ke[:], kr[:])

                    vp = projp.tile([128, 512], F32, tag="big")
                    for kc in range(KC):
                        nc.tensor.matmul(
                            vp[:],
                            xT[:, kc, tl * 128 : (tl + 1) * 128],
                            load_w("v")[:, kc, :],
                            start=(kc == 0),
                            stop=(kc == KC - 1 and not has_bias),
                        )
                    if has_bias:
                        nc.tensor.matmul(
                            vp[:], ones_b[:], bv_sb[:], start=False, stop=True
                        )
                    va_t = va[:, t_c, :].rearrange("p (j h c) -> p j h c", h=2, c=65)
                    vp_t = vp[:].rearrange("p (j h c) -> p j h c", h=2, c=64)
                    if has_mask:
                        nc.scalar.mul(va_t[:, :, :, 0:64], vp_t, m_col)
                        nc.vector.tensor_copy(
                            va_t[:, :, :, 64:65], m_col.broadcast_to((128, OC, 2, 1))
                        )
                    else:
                        nc.scalar.copy(va_t[:, :, :, 0:64], vp_t)

                # ---- phase B: per-pair KV partial accumulation (this quarter) ----
                for j in range(OC if "B" in stages else 0):
                    kvp_t2 = nmp.tile([128, 2, 130], F32, tag="nm")
                    kvp_t = kvp_t2[:, 0, :]
                    for tl in range(TCH):
                        t_c = half * TCH + tl
                        nc.tensor.matmul(
                            kvp_t[:],
                            kf[:, t_c, j * 128 : (j + 1) * 128],
                            va[:, t_c, j * 130 : (j + 1) * 130],
                            start=(tl == 0),
                            stop=(tl == TCH - 1),
                        )
                    nc.vector.tensor_add(kvs32[:, j, :], kvs32[:, j, :], kvp_t[:])

                # ---- phase C: q projection (head-major) + phi ----
                def emit_C(half, t5l, oc, xT=None):
                    t5 = half * T5H + t5l
                    qp = projp.tile([128, 512], F32, tag="big")
                    for kc in range(KC):
                        nc.tensor.matmul(
                            qp[:],
                            load_w("q")[:, kc, oc * 128 : (oc + 1) * 128],
                            xT[:, kc, t5l * 512 : (t5l + 1) * 512],
                            start=(kc == 0),
                            stop=(kc == KC - 1),
                        )
                    b_col = bq_sb[:, oc : oc + 1]
                    qe = ptmp.tile([128, 512], F32, tag="ex")
                    qr = ptmp.tile([128, 512], F32, tag="rl")
                    if has_bias:
                        nc.scalar.activation(qe[:], qp[:], AF.Exp, bias=b_col)
                        nc.scalar.activation(qr[:], qp[:], AF.Relu, bias=b_col)
                    else:
                        nc.scalar.activation(qe[:], qp[:], AF.Exp)
                        nc.scalar.activation(qr[:], qp[:], AF.Relu)
                    nc.vector.tensor_scalar_min(qe[:], qe[:], 1.0)
                    nc.vector.tensor_add(
                        qf[:, oc, t5 * 512 : (t5 + 1) * 512], qe[:], qr[:]
                    )

                if "C" in stages and half < HALVES - 1:
                    for t5l in range(T5H):
                        for oc in range(OC):
                            emit_C(half, t5l, oc, xT=xT)
                else:
                    last_xT = xT

            # ---- finalize kvs (bf16, zero-padded) from kvs32 ----
            if "B" in stages:
                for j in range(OC):
                    kj = kvs32[:, j, :]
                    nc.vector.tensor_copy(kvs[0:64, j, 0:64], kj[0:64, 0:64])
                    nc.vector.tensor_copy(kvs[0:64, j, 128:129], kj[0:64, 64:65])
                    nc.vector.tensor_copy(kvs[64:128, j, 64:128], kj[64:128, 65:129])
                    nc.vector.tensor_copy(kvs[64:128, j, 129:130], kj[64:128, 64:65])

            # ---- phase D: fused num+den + normalize + store ----
            warm = os.environ.get("LK_WARM", "1") == "1"

            def emit_D(t_c, extra_warm=False):
                m_col = mask_sb[:, t_c : t_c + 1]
                if warm:
                    # dense dummy matmul keeps the PE clock-gate at 8/8
                    # through the small-matmul tail (projp is idle here)
                    for _ in range(2 if extra_warm else 1):
                        wp_t = projp.tile([128, 512], F32, tag="big")
                        nc.tensor.matmul(
                            wp_t[:], xT[:, 0, 0:128], load_w("k")[:, 0, :],
                            start=True, stop=True, skip_group_check=True,
                        )
                nms = []
                for jj in range(2):  # two pairs per PSUM bank tile
                    nm2 = nmp.tile([128, 2, 130], F32, tag="nm")
                    for j2 in range(2):
                        nc.tensor.matmul(
                            nm2[:, j2, :],
                            qf[:, jj * 2 + j2, t_c * 128 : (t_c + 1) * 128],
                            kvs[:, jj * 2 + j2, :],
                            start=True,
                            stop=True,
                        )
                    nms.append(nm2)
                rden = rdp.tile([128, 8], F32, tag="rd")
                for jj in range(2):
                    # strided read gathers den cols; max applies the clamp
                    nc.vector.tensor_scalar_max(
                        rden[:].rearrange("p (a b) -> p a b", a=2)[:, jj],
                        nms[jj][:, :, 128:130],
                        1e-6,
                    )
                nc.vector.reciprocal(rden[:], rden[:])
                if has_mask:
                    nc.vector.tensor_scalar_mul(rden[:], rden[:], m_col)
                # two wide muls; recip fed via step-0 broadcast AP (no expand)
                ot = outp.tile([128, HG], F32, tag="ot")
                for jj in range(2):
                    nc.vector.tensor_mul(
                        ot[:, jj * 256 : (jj + 1) * 256].rearrange(
                            "p (a b c) -> p a b c", b=2, c=64
                        ),
                        nms[jj][:, :, 0:128].rearrange("p a (b c) -> p a b c", c=64),
                        rden[:, jj * 4 : (jj + 1) * 4]
                        .rearrange("p (a b) -> p a b", b=2)
                        .unsqueeze(-1)
                        .broadcast_to((128, 2, 2, 64)),
                    )
                nc.sync.dma_start(outd.ap()[t_c * 128 : (t_c + 1) * 128, :], ot[:])

            if "D" in stages:
                d_order = []
                if "C" in stages:
                    # interleave last-quarter q-projection across early D chunks
                    # (dense N=512 bursts keep the HAM clock-gate warm)
                    dq = list(range((HALVES - 1) * TCH))
                    for t5l in range(T5H):
                        for oc in range(OC):
                            d_order.append(("C", t5l, oc))
                            for _ in range(3):
                                if dq:
                                    d_order.append(("D", dq.pop(0), None))
                    for t_c in dq:
                        d_order.append(("D", t_c, None))
                    for t_c in range((HALVES - 1) * TCH, TC):
                        d_order.append(("D", t_c, None))
                else:
                    d_order = [("D", t_c, None) for t_c in range(TC)]
                n_c_left = sum(1 for k, _, _ in d_order if k == "C")
                for kind, a, b2 in d_order:
                    if kind == "C":
                        emit_C(HALVES - 1, a, b2, xT=last_xT)
                        n_c_left -= 1
                    else:
                        emit_D(a, extra_warm=(n_c_left == 0))

            if tpsp_cm is not None:
                tpsp_cm.__exit__(None, None, None)

    nc.compile()
    return nc


_PROGRAM_CACHE = {}


def _get_program(has_bias: bool, has_mask: bool):
    key = (has_bias, has_mask)
    if key not in _PROGRAM_CACHE:
        _PROGRAM_CACHE[key] = _build_program(has_bias, has_mask)
    return _PROGRAM_CACHE[key]


def _prep_inputs(x, mask, Wq, bq, Wk, bk, Wv, bv):
    """Slice + lay out per-core inputs. Core c -> batch c//2, head-group c%2."""
    bf16 = ml_dtypes.bfloat16
    in_maps = []
    for c in range(8):
        bi, hg = c // 2, c % 2
        sl = slice(hg * HG, (hg + 1) * HG)
        in_maps.append(
            {
                "xb": np.ascontiguousarray(x[bi]).astype(np.float32, copy=False),
                "maskb": np.ascontiguousarray(mask[bi]).astype(np.float32, copy=False),
                "wqt": np.ascontiguousarray(Wq[sl, :].T).astype(bf16),
                "wkt": np.ascontiguousarray(Wk[sl, :].T).astype(bf16),
                "wvt": np.ascontiguousarray(Wv[sl, :].T).astype(bf16),
                "bqp": np.ascontiguousarray(bq[sl]).astype(np.float32, copy=False),
                "bkr": np.ascontiguousarray(bk[sl]).astype(bf16).reshape(1, HG),
                "bvr": np.ascontiguousarray(bv[sl]).astype(bf16).reshape(1, HG),
            }
        )
    return in_maps


def kernel(x, mask, Wq, bq, Wk, bk, Wv, bv, n_heads, **run_kwargs):
    x = np.asarray(x)
    mask = np.asarray(mask)
    Wq, bq = np.asarray(Wq), np.asarray(bq)
    Wk, bk = np.asarray(Wk), np.asarray(bk)
    Wv, bv = np.asarray(Wv), np.asarray(bv)
    b, t, d = x.shape
    assert (b, t, d) == (4, T, D) and int(n_heads) == 16, (
        f"kernel hardcoded for (4,{T},{D}) h=16, got {(b, t, d)} h={n_heads}"
    )

    has_bias = bool(np.any(bq) or np.any(bk) or np.any(bv))
    has_mask = not bool(np.all(mask == 1.0))
    nc = _get_program(has_bias, has_mask)
    in_maps = _prep_inputs(x, mask, Wq, bq, Wk, bk, Wv, bv)
    res = run_bass_kernel_spmd(nc, in_maps, core_ids=list(range(8)), **run_kwargs)

    out = np.empty((4, T, D), dtype=np.float32)
    for c in range(8):
        bi, hg = c // 2, c % 2
        out[bi, :, hg * HG : (hg + 1) * HG] = res.results[c]["out"]
    if run_kwargs:
        kernel.last_results = res
    return out


# revision 29
# speedup vs baseline: 1.0123x; 1.0063x over previous
"""Trainium2 Bass kernel for CUDALinearAttention (b=4, t=4096, d=1024, h=16).

Sharding: 8 NeuronCores = 4 batches x 2 head-groups (8 heads / 512 out-dims each).
Each core is fully independent (KV aggregation is per-head); no collectives.

Per-core pipeline (all matmuls bf16, fp32 PSUM accumulation), per t-quarter:
  T: x loaded token-major (split DMAs), cast to bf16 (DVE), transposed to xT
     (d-on-partitions) via PE identity-matmul (8 blocks per PSUM bank) or DMA
     xbar transpose (LK_TMODE=dma).
  A: k/v projections token-major; phi(x)=min(exp(x),1)+relu(x) (exp on ACT
     straight from PSUM, clamp on GpSimd, assemble on DVE), mask folded in;
     v stored per-pair as [v_h0 | m | v_h1 | m] (130-wide blocks).
  B: per head pair j one matmul chain over the quarter's t: lhsT = kf pair
     cols [128,128], rhs = va pair block [128,130] -> kv of both heads in
     row-halves, z in col 64 (garbage halves never read); accumulated across
     quarters into SBUF (kvs32), so the small-matmul work stays inside the
     dense (HAM-warm) region.
  C: q projection head-major (W stationary, xT moving): qfT[o,t] -- already
     K(=hd)-major for num/den.
Then kvs32 is finalized zero-padded into kvs (so downstream matmuls contract
K=128 from base partition 0; row-group-64 operands crash hardware), and
  D: one matmul per pair/chunk: rhs = kvs[:,j,:] = [kv_h0|kv_h1|z0|z1]
     [128,130] -> cols 0..127 = num both heads, 128/129 = den; then
     out = num * recip(max(den,1e-6)) * mask on DVE; DMA out.
"""

import os
import sys

sys.path.insert(0, "/opt/trn_rl_repo")

import numpy as np
import ml_dtypes

import concourse.bass as bass
import concourse.tile as tile
from concourse import bacc, mybir
from concourse.bass_utils import run_bass_kernel_spmd
from concourse.masks import make_identity

F32 = mybir.dt.float32
BF16 = mybir.dt.bfloat16
AF = mybir.ActivationFunctionType
ALU = mybir.AluOpType

T = 4096
D = 1024
HG = 512  # per-core output dims (8 heads x 64)
KC = 8  # contraction chunks of 128 over D
TC = 32  # token chunks of 128
OC = 4  # output-dim chunks of 128 within HG (= head pairs)
HALVES = 4  # t mega-chunks (xT quarter double-buffered)
TCH = TC // HALVES
T5H = (T // 512) // HALVES


def _build_program(has_bias: bool, has_mask: bool):
    stages = os.environ.get("LK_STAGES", "TABCD")
    tmode = os.environ.get("LK_TMODE", "pe")
    nc = bacc.Bacc("TRN2", target_bir_lowering=False, debug=False)

    xb = nc.dram_tensor("xb", [T, D], F32, kind="ExternalInput")
    maskb = nc.dram_tensor("maskb", [T], F32, kind="ExternalInput")
    wqt = nc.dram_tensor("wqt", [D, HG], BF16, kind="ExternalInput")
    wkt = nc.dram_tensor("wkt", [D, HG], BF16, kind="ExternalInput")
    wvt = nc.dram_tensor("wvt", [D, HG], BF16, kind="ExternalInput")
    bqp = nc.dram_tensor("bqp", [HG], F32, kind="ExternalInput")
    bkr = nc.dram_tensor("bkr", [1, HG], BF16, kind="ExternalInput")
    bvr = nc.dram_tensor("bvr", [1, HG], BF16, kind="ExternalInput")
    outd = nc.dram_tensor("out", [T, HG], F32, kind="ExternalOutput")

    with tile.TileContext(nc) as tc:
        with (
            tc.tile_pool(name="const", bufs=1) as constp,
            tc.tile_pool(name="wp", bufs=1) as wp,
            tc.tile_pool(name="xTp", bufs=2) as xTp,
            tc.tile_pool(name="kfp", bufs=1) as kfp,
            tc.tile_pool(name="vap", bufs=1) as vap,
            tc.tile_pool(name="qfp", bufs=1) as qfp,
            tc.tile_pool(name="kvsp", bufs=1) as kvsp,
            tc.tile_pool(name="stage", bufs=4) as stage,
            tc.tile_pool(name="ptmp", bufs=3) as ptmp,
            tc.tile_pool(name="outp", bufs=3) as outp,
            tc.tile_pool(name="rdp", bufs=2) as rdp,
            tc.tile_pool(
                name="projp", bufs=4 if tmode != "pe" else 3, space="PSUM"
            ) as projp,
            tc.tile_pool(
                name="nmp", bufs=4 if tmode != "pe" else 3, space="PSUM"
            ) as nmp,
        ):
            tpsp_cm = None
            tpsp = None
            if tmode == "pe":
                tpsp_cm = tc.tile_pool(name="tpsp", bufs=2, space="PSUM")
                tpsp = tpsp_cm.__enter__()

            # pre-issue the first x tiles before anything else (startup latency)
            xs_pre = []
            for i in range(4):
                xsp = stage.tile([128, D], F32, tag="xs")
                r = slice(i * 128, (i + 1) * 128)
                nc.sync.dma_start(xsp[:, 0:512], xb.ap()[r, 0:512])
                nc.sync.dma_start(xsp[:, 512:1024], xb.ap()[r, 512:1024])
                xs_pre.append(xsp)

            # ---- constants ----
            ident = constp.tile([128, 128], BF16)
            make_identity(nc, ident[:])
            mask_sb = constp.tile([128, TC], F32)
            nc.sync.dma_start(mask_sb[:], maskb.ap().rearrange("(a p) -> p a", p=128))
            bq_sb = constp.tile([128, OC], F32)
            nc.sync.dma_start(bq_sb[:], bqp.ap().rearrange("(a p) -> p a", p=128))
            eps_sb = constp.tile([128, 1], F32)
            nc.vector.memset(eps_sb[:], 1e-6)
            if has_bias:
                ones_b = constp.tile([1, 128], BF16)
                nc.vector.memset(ones_b[:], 1.0)
                bk_sb = constp.tile([1, HG], BF16)
                nc.sync.dma_start(bk_sb[:], bkr.ap())
                bv_sb = constp.tile([1, HG], BF16)
                nc.sync.dma_start(bv_sb[:], bvr.ap())

            # ---- weights (host pre-transposed to [D, HG]); DMAs emitted
            # lazily so the first x tiles win the DMA queues ----
            w_sb = {}
            w_dram = {"q": wqt, "k": wkt, "v": wvt}

            def load_w(name):
                if name not in w_sb:
                    w = wp.tile([128, KC, HG], BF16, tag=f"w{name}")
                    nc.sync.dma_start(
                        w[:], w_dram[name].ap().rearrange("(kc p) n -> p kc n", p=128)
                    )
                    w_sb[name] = w
                return w_sb[name]

            # ---- big persistent activations ----
            kf = kfp.tile([128, TC, HG], BF16)
            va = vap.tile([128, TC, OC * 130], BF16)
            qf = qfp.tile([128, OC, T], BF16)
            kvs32 = kvsp.tile([128, OC, 130], F32, tag="kvs32")
            nc.vector.memset(kvs32[:], 0.0)
            # kvs[:, j, :] = [kv_h0 (rows 0-63) | kv_h1 (rows 64-127) | z0 | z1],
            # complementary rows zero
            kvs = kvsp.tile([128, OC, 130], BF16)
            nc.vector.memset(kvs[:], 0.0)
            if not has_mask:
                # ones columns of va are the constant 1.0 mask; set once
                va_ones = va[:].rearrange("p t (j h c) -> p t j h c", h=2, c=65)
                nc.vector.memset(va_ones[:, :, :, :, 64:65], 1.0)

            for half in range(HALVES):
                xT = xTp.tile([128, KC, T // HALVES], BF16, tag="xT")

                # ---- phase T: load + cast + transpose ----
                for tl in range(TCH if "T" in stages else 0):
                    t_c = half * TCH + tl
                    if t_c < 4:
                        xs = xs_pre[t_c]
                    else:
                        xs = stage.tile([128, D], F32, tag="xs")
                        r = slice(t_c * 128, (t_c + 1) * 128)
                        nc.sync.dma_start(xs[:], xb.ap()[r, :])
                    xc = stage.tile([128, D], BF16, tag="xc")
                    nc.vector.tensor_copy(xc[:, 0:512], xs[:, 0:512])
                    nc.vector.tensor_copy(xc[:, 512:1024], xs[:, 512:1024])
                    if tmode == "dma1":
                        nc.sync.dma_start_transpose(
                            xT[:, :, tl * 128 : (tl + 1) * 128], xc[:]
                        )
                    elif tmode == "dma":
                        for kc in range(KC):
                            nc.sync.dma_start_transpose(
                                xT[:, kc, tl * 128 : (tl + 1) * 128],
                                xc[:, kc * 128 : (kc + 1) * 128],
                            )
                    else:
                        tp = tpsp.tile([128, KC, 128], BF16, tag="tps")
                        for kc in range(KC):
                            nc.tensor.matmul(
                                tp[:, kc, :],
                                xc[:, kc * 128 : (kc + 1) * 128],
                                ident[:],
                                is_transpose=True,
                                start=(kc == 0),
                                stop=(kc == KC - 1),
                            )
                        dst = xT[:, :, tl * 128 : (tl + 1) * 128]
                        if tl % 2 == 0:
                            nc.vector.tensor_copy(dst, tp[:])
                        else:
                            nc.scalar.copy(dst, tp[:])

                # ---- phase A: k/v projections (token-major) + phi/mask ----
                for tl in range(TCH if "A" in stages else 0):
                    t_c = half * TCH + tl
                    m_col = mask_sb[:, t_c : t_c + 1]

                    kp = projp.tile([128, 512], F32, tag="big")
                    for kc in range(KC):
                        nc.tensor.matmul(
                            kp[:],
                            xT[:, kc, tl * 128 : (tl + 1) * 128],
                            load_w("k")[:, kc, :],
                            start=(kc == 0),
                            stop=(kc == KC - 1 and not has_bias),
                        )
                    if has_bias:
                        nc.tensor.matmul(
                            kp[:], ones_b[:], bk_sb[:], start=False, stop=True
                        )
                    # phi(x) = min(exp(x), 1) + relu(x); exp is safe: min(inf,1)=1
                    ke = ptmp.tile([128, 512], F32, tag="ex")
                    nc.scalar.activation(ke[:], kp[:], AF.Exp)
                    nc.vector.tensor_scalar_min(ke[:], ke[:], 1.0)
                    kr = ptmp.tile([128, 512], F32, tag="rl")
                    if has_mask:
                        # relu(k * m) == m * relu(k) for m >= 0
                        nc.scalar.activation(kr[:], kp[:], AF.Relu, scale=m_col)
                        nc.vector.scalar_tensor_tensor(
                            kf[:, t_c, :], ke[:], m_col, kr[:],
                            op0=ALU.mult, op1=ALU.add,
                        )
                    else:
                        nc.scalar.activation(kr[:], kp[:], AF.Relu)
                        nc.vector.tensor_add(kf[:, t_c, :], ke[:], kr[:])

                    vp = projp.tile([128, 512], F32, tag="big")
                    for kc in range(KC):
                        nc.tensor.matmul(
                            vp[:],
                            xT[:, kc, tl * 128 : (tl + 1) * 128],
                            load_w("v")[:, kc, :],
                            start=(kc == 0),
                            stop=(kc == KC - 1 and not has_bias),
                        )
                    if has_bias:
                        nc.tensor.matmul(
                            vp[:], ones_b[:], bv_sb[:], start=False, stop=True
                        )
                    va_t = va[:, t_c, :].rearrange("p (j h c) -> p j h c", h=2, c=65)
                    vp_t = vp[:].rearrange("p (j h c) -> p j h c", h=2, c=64)
                    if has_mask:
                        nc.scalar.mul(va_t[:, :, :, 0:64], vp_t, m_col)
                        nc.vector.tensor_copy(
                            va_t[:, :, :, 64:65], m_col.broadcast_to((128, OC, 2, 1))
                        )
                    else:
                        nc.scalar.copy(va_t[:, :, :, 0:64], vp_t)

                # ---- phase B: per-pair KV partial accumulation (this quarter) ----
                for j in range(OC if "B" in stages else 0):
                    kvp_t2 = nmp.tile([128, 2, 130], F32, tag="nm")
                    kvp_t = kvp_t2[:, 0, :]
                    for tl in range(TCH):
                        t_c = half * TCH + tl
                        nc.tensor.matmul(
                            kvp_t[:],
                            kf[:, t_c, j * 128 : (j + 1) * 128],
                            va[:, t_c, j * 130 : (j + 1) * 130],
                            start=(tl == 0),
                            stop=(tl == TCH - 1),
                        )
                    nc.vector.tensor_add(kvs32[:, j, :], kvs32[:, j, :], kvp_t[:])

                # ---- phase C: q projection (head-major) + phi ----
                def emit_C(half, t5l, oc, xT=None):
                    t5 = half * T5H + t5l
                    qp = projp.tile([128, 512], F32, tag="big")
                    for kc in range(KC):
                        nc.tensor.matmul(
                            qp[:],
                            load_w("q")[:, kc, oc * 128 : (oc + 1) * 128],
                            xT[:, kc, t5l * 512 : (t5l + 1) * 512],
                            start=(kc == 0),
                            stop=(kc == KC - 1),
                        )
                    b_col = bq_sb[:, oc : oc + 1]
                    qe = ptmp.tile([128, 512], F32, tag="ex")
                    qr = ptmp.tile([128, 512], F32, tag="rl")
                    if has_bias:
                        nc.scalar.activation(qe[:], qp[:], AF.Exp, bias=b_col)
                        nc.scalar.activation(qr[:], qp[:], AF.Relu, bias=b_col)
                    else:
                        nc.scalar.activation(qe[:], qp[:], AF.Exp)
                        nc.scalar.activation(qr[:], qp[:], AF.Relu)
                    nc.vector.tensor_scalar_min(qe[:], qe[:], 1.0)
                    nc.vector.tensor_add(
                        qf[:, oc, t5 * 512 : (t5 + 1) * 512], qe[:], qr[:]
                    )

                if "C" in stages and half < HALVES - 1:
                    for t5l in range(T5H):
                        for oc in range(OC):
                            emit_C(half, t5l, oc, xT=xT)
                else:
                    last_xT = xT

            # ---- finalize kvs (bf16, zero-padded) from kvs32 ----
            if "B" in stages:
                for j in range(OC):
                    kj = kvs32[:, j, :]
                    nc.vector.tensor_copy(kvs[0:64, j, 0:64], kj[0:64, 0:64])
                    nc.vector.tensor_copy(kvs[0:64, j, 128:129], kj[0:64, 64:65])
                    nc.vector.tensor_copy(kvs[64:128, j, 64:128], kj[64:128, 65:129])
                    nc.vector.tensor_copy(kvs[64:128, j, 129:130], kj[64:128, 64:65])

            # ---- phase D: fused num+den + normalize + store ----
            warm = os.environ.get("LK_WARM", "1") == "1"

            def emit_D(t_c, extra_warm=False):
                m_col = mask_sb[:, t_c : t_c + 1]
                if warm:
                    # dense dummy matmul keeps the PE clock-gate at 8/8
                    # through the small-matmul tail (projp is idle here)
                    for _ in range(1):
                        wp_t = projp.tile([128, 512], F32, tag="big")
                        nc.tensor.matmul(
                            wp_t[:], xT[:, 0, 0:128], load_w("k")[:, 0, :],
                            start=True, stop=True, skip_group_check=True,
                        )
                nms = []
                for jj in range(2):  # two pairs per PSUM bank tile
                    nm2 = nmp.tile([128, 2, 130], F32, tag="nm")
                    for j2 in range(2):
                        nc.tensor.matmul(
                            nm2[:, j2, :],
                            qf[:, jj * 2 + j2, t_c * 128 : (t_c + 1) * 128],
                            kvs[:, jj * 2 + j2, :],
                            start=True,
                            stop=True,
                        )
                    nms.append(nm2)
                rden = rdp.tile([128, 8], F32, tag="rd")
                for jj in range(2):
                    # strided read gathers den cols; max applies the clamp
                    nc.vector.tensor_scalar_max(
                        rden[:].rearrange("p (a b) -> p a b", a=2)[:, jj],
                        nms[jj][:, :, 128:130],
                        1e-6,
                    )
                nc.vector.reciprocal(rden[:], rden[:])
                if has_mask:
                    nc.vector.tensor_scalar_mul(rden[:], rden[:], m_col)
                # two wide muls; recip fed via step-0 broadcast AP (no expand)
                ot = outp.tile([128, HG], F32, tag="ot")
                for jj in range(2):
                    nc.vector.tensor_mul(
                        ot[:, jj * 256 : (jj + 1) * 256].rearrange(
                            "p (a b c) -> p a b c", b=2, c=64
                        ),
                        nms[jj][:, :, 0:128].rearrange("p a (b c) -> p a b c", c=64),
                        rden[:, jj * 4 : (jj + 1) * 4]
                        .rearrange("p (a b) -> p a b", b=2)
                        .unsqueeze(-1)
                        .broadcast_to((128, 2, 2, 64)),
                    )
                nc.sync.dma_start(outd.ap()[t_c * 128 : (t_c + 1) * 128, :], ot[:])

            if "D" in stages:
                d_order = []
                if "C" in stages:
                    # interleave last-quarter q-projection across early D chunks
                    # (dense N=512 bursts keep the HAM clock-gate warm)
                    dq = list(range((HALVES - 1) * TCH))
                    for t5l in range(T5H):
                        for oc in range(OC):
                            d_order.append(("C", t5l, oc))
                            for _ in range(3):
                                if dq:
                                    d_order.append(("D", dq.pop(0), None))
                    for t_c in dq:
                        d_order.append(("D", t_c, None))
                    for t_c in range((HALVES - 1) * TCH, TC):
                        d_order.append(("D", t_c, None))
                else:
                    d_order = [("D", t_c, None) for t_c in range(TC)]
                n_c_left = sum(1 for k, _, _ in d_order if k == "C")
                for kind, a, b2 in d_order:
                    if kind == "C":
                        emit_C(HALVES - 1, a, b2, xT=last_xT)
                        n_c_left -= 1
                    else:
                        emit_D(a, extra_warm=(n_c_left == 0))

            if tpsp_cm is not None:
                tpsp_cm.__exit__(None, None, None)

    nc.compile()
    return nc


_PROGRAM_CACHE = {}


def _get_program(has_bias: bool, has_mask: bool):
    key = (has_bias, has_mask)
    if key not in _PROGRAM_CACHE:
        _PROGRAM_CACHE[key] = _build_program(has_bias, has_mask)
    return _PROGRAM_CACHE[key]


def _prep_inputs(x, mask, Wq, bq, Wk, bk, Wv, bv):
    """Slice + lay out per-core inputs. Core c -> batch c//2, head-group c%2."""
    bf16 = ml_dtypes.bfloat16
    in_maps = []
    for c in range(8):
        bi, hg = c // 2, c % 2
        sl = slice(hg * HG, (hg + 1) * HG)
        in_maps.append(
            {
                "xb": np.ascontiguousarray(x[bi]).astype(np.float32, copy=False),
                "maskb": np.ascontiguousarray(mask[bi]).astype(np.float32, copy=False),
                "wqt": np.ascontiguousarray(Wq[sl, :].T).astype(bf16),
                "wkt": np.ascontiguousarray(Wk[sl, :].T).astype(bf16),
                "wvt": np.ascontiguousarray(Wv[sl, :].T).astype(bf16),
                "bqp": np.ascontiguousarray(bq[sl]).astype(np.float32, copy=False),
                "bkr": np.ascontiguousarray(bk[sl]).astype(bf16).reshape(1, HG),
                "bvr": np.ascontiguousarray(bv[sl]).astype(bf16).reshape(1, HG),
            }
        )
    return in_maps


def kernel(x, mask, Wq, bq, Wk, bk, Wv, bv, n_heads, **run_kwargs):
    x = np.asarray(x)
    mask = np.asarray(mask)
    Wq, bq = np.asarray(Wq), np.asarray(bq)
    Wk, bk = np.asarray(Wk), np.asarray(bk)
    Wv, bv = np.asarray(Wv), np.asarray(bv)
    b, t, d = x.shape
    assert (b, t, d) == (4, T, D) and int(n_heads) == 16, (
        f"kernel hardcoded for (4,{T},{D}) h=16, got {(b, t, d)} h={n_heads}"
    )

    has_bias = bool(np.any(bq) or np.any(bk) or np.any(bv))
    has_mask = not bool(np.all(mask == 1.0))
    nc = _get_program(has_bias, has_mask)
    in_maps = _prep_inputs(x, mask, Wq, bq, Wk, bk, Wv, bv)
    res = run_bass_kernel_spmd(nc, in_maps, core_ids=list(range(8)), **run_kwargs)

    out = np.empty((4, T, D), dtype=np.float32)
    for c in range(8):
        bi, hg = c // 2, c % 2
        out[bi, :, hg * HG : (hg + 1) * HG] = res.results[c]["out"]
    if run_kwargs:
        kernel.last_results = res
    return out
